# revision 11
# baseline (speedup 1.0000x reference)
"""Multi-Head Latent Attention (MLA) Trainium2 kernel.

Problem: B=2, T=2048, D=2048, H=16 heads, HD=128, LAT=512, RD=64, CD=64.
Sharding: 8 cores = (batch 2) x (head-group 4). Each core handles one
batch and 4 heads: q/k/v/out projections sharded by head.

The latent kv down-projection is never materialized on device: the
per-head k-content and v projections are folded with the down-
projection on the host (Wk_eff = Wku[heads] @ Wkv_k, Wv_eff =
Wvu[heads] @ Wkv_v), so each core computes its heads' K/V directly
from x.  Per core that is 196.5k PE cycles instead of 311k for the
replicated-latent scheme, and needs no cross-core collective (a 4-core
AllGather of the latent measures ~190us here, far too slow to hide).

Per-core data layouts (all "T" suffixed = transposed, feature-major):
  xT      [D=2048, T=2048]  bf16   x[b].T
  qT_s    per head [HD=128, T]     (scale 1/sqrt(HD) folded into Wq, RoPE applied)
  kT_s    per head [HD=128, T]     rows 0:64 content (direct), 64:128 shared rope key
  v_s     per tk-tile [128, 4*HD=512]  v in natural [t, e] layout (direct)
  scores  S^T [tk, tq] in PSUM  ->  exp -> P^T bf16 in SBUF
  pa      fp32 running sum of P^T tiles (DVE) -> single ones-matmul denom
  outT    [HD, T] accumulated in PSUM, normalized by softmax denom
  yT      [D, T] fp32 partial output (no biases; host adds Wo@bvu_eff + bo)
"""

import sys
import numpy as np
import ml_dtypes

sys.path.insert(0, "/opt/trn_rl_repo")

import concourse.bass as bass
import concourse.bacc as bacc
import concourse.tile as tile
import concourse.mybir as mybir
from concourse.bass_utils import run_bass_kernel_spmd

B, T, D = 2, 2048, 2048
H, HD, LAT, RD = 16, 128, 512, 64
CD = HD - RD
THETA = 10000.0
NH = 4            # heads per core
NCORES = 8
TQ = 512          # tq block (matmul moving free dim)
TKT = 128         # tk tile (stationary)
NEG = -10000.0    # additive causal mask

BF = mybir.dt.bfloat16
F32 = mybir.dt.float32

def build_nc(loop=1):
    nc = bacc.Bacc("TRN2", target_bir_lowering=False, debug=False)

    xT_d = nc.declare_dram_parameter("xT", [D, T], BF, isOutput=False)
    wqT_d = nc.declare_dram_parameter("wqT", [D, NH * HD], BF, isOutput=False)
    wkT_d = nc.declare_dram_parameter("wkT", [D, NH * CD], BF, isOutput=False)
    wvT_d = nc.declare_dram_parameter("wvT", [D, NH * HD], BF, isOutput=False)
    wkrT_d = nc.declare_dram_parameter("wkrT", [D, RD], BF, isOutput=False)
    # packed per-partition biases, every used slice starting at partition 0:
    # cols 0..3 bq(head), 12 bkr, 13..16 bq-rope(head), 17..20 bk_eff(head)
    bias_d = nc.declare_dram_parameter("biases", [128, 21], F32, isOutput=False)
    woT_d = nc.declare_dram_parameter("woT", [NH * HD, D], BF, isOutput=False)
    cos_d = nc.declare_dram_parameter("cosT", [RD, T], F32, isOutput=False)
    sin_d = nc.declare_dram_parameter("sinT", [RD, T], F32, isOutput=False)
    mask_d = nc.declare_dram_parameter("maskneg", [TKT, TKT], F32, isOutput=False)
    ones_d = nc.declare_dram_parameter("ones_tk", [TKT, 1], BF, isOutput=False)
    yT_d = nc.declare_dram_parameter("yT", [D, T], F32, isOutput=True)

    ND = D // 128          # 16 d-tiles
    NJ = T // TQ           # 4 tq blocks
    NKT = T // TKT         # 16 tk tiles

    with tile.TileContext(nc) as tc:
        body(nc, tc, loop, locals())
    nc.compile()
    return nc


def body(nc, tc, loop, dr):
    xT_d, wqT_d, bias_d = dr["xT_d"], dr["wqT_d"], dr["bias_d"]
    wkT_d, wvT_d, wkrT_d = dr["wkT_d"], dr["wvT_d"], dr["wkrT_d"]
    woT_d = dr["woT_d"]
    cos_d, sin_d, mask_d, ones_d, yT_d = (
        dr["cos_d"], dr["sin_d"], dr["mask_d"], dr["ones_d"], dr["yT_d"])
    ND, NJ, NKT = dr["ND"], dr["NJ"], dr["NKT"]
    AExp = mybir.ActivationFunctionType.Exp

    from contextlib import ExitStack

    with ExitStack() as ctx:
        # ---- persistent pools (live across phases) ----
        p_per = ctx.enter_context(tc.tile_pool(name="per", bufs=1))
        p_psum = ctx.enter_context(tc.tile_pool(name="psum", bufs=6, space="PSUM"))
        p_psd = ctx.enter_context(tc.tile_pool(name="psd", bufs=2, space="PSUM"))

        # persistent SBUF tensors
        qT_s = [p_per.tile([128, T], BF, name=f"qT{h}", tag=f"qT{h}") for h in range(NH)]
        kT_s = [p_per.tile([128, T], BF, name=f"kT{h}", tag=f"kT{h}") for h in range(NH)]
        v_s = [p_per.tile([128, NH * HD], BF, name=f"v{m}", tag=f"v{m}") for m in range(NKT)]
        krT_s = p_per.tile([RD, T], BF, tag="krT")
        mask_s = p_per.tile([TKT, TKT], F32, tag="mask")
        ones_s = p_per.tile([TKT, 1], BF, tag="ones")
        bias_s = p_per.tile([128, 21], F32, tag="bias")
        bq_s = [bias_s[:, i:i + 1] for i in range(NH)]
        bkr_s = bias_s[0:RD, 12:13]
        bqr_s = [bias_s[0:RD, 13 + h:14 + h] for h in range(NH)]
        bk_s = [bias_s[0:CD, 17 + h:18 + h] for h in range(NH)]

        for _ in range(loop):
            with tc.tile_pool(name="ph1", bufs=1) as p_x:
                xT_s = [p_x.tile([128, T], BF, name=f"xt{i}", tag=f"xt{i}") for i in range(ND)]
                cos_s = p_x.tile([RD, T], F32, tag="cos")
                sin_s = p_x.tile([RD, T], F32, tag="sin")

                # ---------- phase 1a: direct K (content+rope) and V ----------
                wqT_s = [p_x.tile([128, NH * HD], BF, name=f"wq{i}", tag=f"wq{i}")
                         for i in range(ND)]
                with tc.tile_pool(name="w1a", bufs=1) as p_w:
                    wkT_s = [p_w.tile([128, NH * CD], BF, name=f"wk{i}", tag=f"wk{i}")
                             for i in range(ND)]
                    wvT_s = [p_w.tile([128, NH * HD], BF, name=f"wv{i}", tag=f"wv{i}")
                             for i in range(ND)]
                    wkrT_s = [p_w.tile([128, RD], BF, name=f"wkr{i}", tag=f"wkr{i}")
                              for i in range(ND)]
                    # interleave loads in the order phase-1a consumes
                    # them; wq is prefetched at the tail so phase 1b
                    # starts without a DMA stall.
                    for i in range(ND):
                        nc.sync.dma_start(wkT_s[i][:],
                                          wkT_d[i * 128:(i + 1) * 128, :])
                        nc.sync.dma_start(xT_s[i][:],
                                          xT_d[i * 128:(i + 1) * 128, :])
                        if i == 1:
                            nc.sync.dma_start(bias_s[:], bias_d[:, :])
                            nc.sync.dma_start(ones_s[:], ones_d[:, :])
                    for i in range(ND):
                        nc.sync.dma_start(wkrT_s[i][:],
                                          wkrT_d[i * 128:(i + 1) * 128, :])
                    nc.sync.dma_start(cos_s[:], cos_d[:, :])
                    nc.sync.dma_start(sin_s[:], sin_d[:, :])
                    for i in range(ND):
                        nc.sync.dma_start(wvT_s[i][:],
                                          wvT_d[i * 128:(i + 1) * 128, :])
                    nc.sync.dma_start(mask_s[:], mask_d[:, :])
                    for i in range(ND):
                        nc.sync.dma_start(wqT_s[i][:],
                                          wqT_d[i * 128:(i + 1) * 128, :])

                    with tc.tile_pool(name="krtmp", bufs=3) as p_kr:
                        # k content direct: heads in pairs (2 x 64 rows
                        # per 128-row matmul), accumulated over d tiles.
                        for p in range(NH // 2):
                            pss = [p_psum.tile([128, TQ], F32,
                                                name=f"pkc{_j}", tag="mm")
                                   for _j in range(NJ)]
                            for d in range(ND):
                                for j in range(NJ):
                                    nc.tensor.matmul(
                                        pss[j][:],
                                        wkT_s[d][:, p * 128:(p + 1) * 128],
                                        xT_s[d][:, j * TQ:(j + 1) * TQ],
                                        start=(d == 0), stop=(d == ND - 1))
                            for j in range(NJ):
                                for hh in range(2):
                                    h = 2 * p + hh
                                    nc.vector.tensor_scalar_add(
                                        kT_s[h][0:CD, j * TQ:(j + 1) * TQ],
                                        pss[j][hh * CD:(hh + 1) * CD, :],
                                        bk_s[h])
                        # rope key: kr^T [64, t], RoPE -> krT_s (bf16)
                        pss = [p_psum.tile([RD, TQ], F32,
                                            name=f"pkr{_j}", tag="mm")
                               for _j in range(NJ)]
                        for d in range(ND):
                            for j in range(NJ):
                                nc.tensor.matmul(
                                    pss[j][:], wkrT_s[d][:, :],
                                    xT_s[d][:, j * TQ:(j + 1) * TQ],
                                    start=(d == 0), stop=(d == ND - 1))
                        for j in range(NJ):
                            _rope(nc, p_kr, pss[j][:], bkr_s, cos_s, sin_s, j,
                                  krT_s[:, j * TQ:(j + 1) * TQ])
                        # shared rope rows into each head's k
                        for h in range(NH):
                            nc.vector.tensor_copy(kT_s[h][CD:HD, :], krT_s[:])

                        # v direct: [tk-tile, e] natural layout;
                        # stationary = xT column tile, moving = wvT.
                        for m in range(NKT):
                            ps = p_psum.tile([128, NH * HD], F32, tag="mm")
                            for d in range(ND):
                                nc.tensor.matmul(
                                    ps[:],
                                    xT_s[d][:, m * 128:(m + 1) * 128],
                                    wvT_s[d][:],
                                    start=(d == 0), stop=(d == ND - 1))
                            nc.vector.tensor_copy(v_s[m][:], ps[:])

                # ---------- phase 1b: q projection (+ rope on last 64 dims) --
                if True:
                    with tc.tile_pool(name="qtmp", bufs=3) as p_qr:
                        for h in range(NH):
                            pss = [p_psum.tile([128, TQ], F32,
                                                name=f"pq{_j}", tag="mm")
                                   for _j in range(NJ)]
                            for d in range(ND):
                                for j in range(NJ):
                                    nc.tensor.matmul(
                                        pss[j][:],
                                        wqT_s[d][:, h * 128:(h + 1) * 128],
                                        xT_s[d][:, j * TQ:(j + 1) * TQ],
                                        start=(d == 0), stop=(d == ND - 1))
                            for j in range(NJ):
                                # content rows 0:64 -> bias add, cast bf16
                                nc.vector.tensor_scalar_add(
                                    qT_s[h][0:CD, j * TQ:(j + 1) * TQ],
                                    pss[j][0:CD, :], bq_s[h][0:CD, :])
                                # rope rows 64:128
                                _rope(nc, p_qr, pss[j][CD:HD, :],
                                      bqr_s[h],
                                      cos_s, sin_s, j,
                                      qT_s[h][CD:HD, j * TQ:(j + 1) * TQ])

            # ---------- phase 2: attention + out proj ----------
            with tc.tile_pool(name="ph2", bufs=1) as p_2:
                woT_s = [p_2.tile([128, D], BF, name=f"wo{i}", tag=f"wo{i}")
                         for i in range(NH)]
                for i in range(NH):
                    nc.sync.dma_start(woT_s[i][:], woT_d[i * 128:(i + 1) * 128, :])
                aoT_s = [p_2.tile([128, T], BF, name=f"ao{h}", tag=f"ao{h}") for h in range(NH)]

                # ---------- attention (1-step software pipeline),
                # j-outer so each tq column's out-projection can be
                # interleaved as soon as all 4 heads of that column are
                # normalized; this fills PE bubbles and shrinks the tail.
                with tc.tile_pool(name="pT", bufs=1) as p_pT, \
                     tc.tile_pool(name="att", bufs=4) as p_att, \
                     tc.tile_pool(name="yout", bufs=4) as p_y:

                    def finish(h, j, pts, pa):
                        ntk = 4 * (j + 1)
                        # columns < lo(kk) of a diagonal tile are fully
                        # masked; skip them in every chain (tile kk=0 is
                        # always full-width, so start=True covers the bank)
                        lo = lambda kk: 128 * max(0, kk - 4 * j)
                        # denominator: single ones^T @ sum_kk(P^T) matmul
                        # (pa holds the fp32 DVE-accumulated P sum)
                        pa_bf = p_att.tile([TKT, TQ], BF, tag="pabf")
                        nc.vector.tensor_copy(pa_bf[:], pa[:])
                        pd = p_psd.tile([1, TQ], F32, tag="den")
                        nc.tensor.matmul(pd[:], ones_s[:], pa_bf[:],
                                         start=True, stop=True)
                        # out^T accumulation
                        po = p_psum.tile([HD, TQ], F32, tag="mm")
                        for kk in range(ntk):
                            nc.tensor.matmul(
                                po[:, lo(kk):],
                                v_s[kk][:, h * HD:(h + 1) * HD],
                                pts[kk][:, lo(kk):],
                                start=(kk == 0), stop=(kk == ntk - 1))
                        # normalize: out^T * (1/denom) with the fp32
                        # reciprocal broadcast to 128 partitions on GpSimd.
                        rec = p_att.tile([1, TQ], F32, tag="rec")
                        nc.vector.reciprocal(rec[:], pd[:])
                        bc = p_att.tile([128, TQ], F32, tag="bc")
                        nc.gpsimd.partition_broadcast(bc[:], rec[:],
                                                      channels=128)
                        nc.vector.tensor_mul(
                            aoT_s[h][:, j * TQ:(j + 1) * TQ], po[:], bc[:])

                    def out_proj_col(j):
                        for eo in range(D // 128):
                            ps = p_psum.tile([128, TQ], F32, tag="mm")
                            for dl in range(NH):
                                nc.tensor.matmul(
                                    ps[:],
                                    woT_s[dl][:, eo * 128:(eo + 1) * 128],
                                    aoT_s[dl][:, j * TQ:(j + 1) * TQ],
                                    start=(dl == 0), stop=(dl == NH - 1))
                            ys = p_y.tile([128, TQ], F32, tag="y")
                            nc.vector.tensor_copy(ys[:], ps[:])
                            nc.sync.dma_start(
                                yT_d[eo * 128:(eo + 1) * 128,
                                     j * TQ:(j + 1) * TQ], ys[:])

                    prev = None
                    for j in range(NJ):
                        for h in range(NH):
                            ntk = 4 * (j + 1)
                            pts = [p_pT.tile([TKT, TQ], BF, name=f"pT{kk}",
                                             tag=f"pT{kk}", bufs=2)
                                   for kk in range(ntk)]
                            pa = p_pT.tile([TKT, TQ], F32, name="paAcc",
                                           tag="paAcc", bufs=2)
                            for kk in range(ntk):
                                m = kk - 4 * j
                                lo = 128 * max(0, m)
                                ps = p_psum.tile([TKT, TQ], F32, tag="mm")
                                nc.tensor.matmul(
                                    ps[:, lo:],
                                    kT_s[h][:, kk * TKT:(kk + 1) * TKT],
                                    qT_s[h][:, j * TQ + lo:(j + 1) * TQ],
                                    start=True, stop=True)
                                if m >= 0:
                                    # triangular corner only; cols < lo are
                                    # skipped, cols >= lo+128 are all-pass
                                    nc.vector.tensor_add(
                                        ps[:, lo:lo + TKT], ps[:, lo:lo + TKT],
                                        mask_s[:])
                                nc.scalar.activation(
                                    pts[kk][:, lo:], ps[:, lo:], AExp)
                                # running fp32 sum of P tiles for the
                                # softmax denominator (masked cols excluded)
                                if kk == 0:
                                    nc.vector.tensor_copy(pa[:], pts[0][:])
                                else:
                                    nc.vector.tensor_add(
                                        pa[:, lo:], pa[:, lo:],
                                        pts[kk][:, lo:])
                            if prev is not None:
                                finish(*prev)
                                if prev[0] == NH - 1:
                                    out_proj_col(prev[1])
                            prev = (h, j, pts, pa)
                    finish(*prev)
                    out_proj_col(NJ - 1)


def _rope(nc, pool, ps_ap, bias_ap, cos_s, sin_s, j, out_ap):
    """RoPE on a [64, TQ] PSUM block (rotate-half, RD=64), bf16 out.
    out[0:32] = y[0:32]*cos[0:32] - y[32:64]*sin[0:32]
    out[32:64] = y[32:64]*cos[32:64] + y[0:32]*sin[32:64],  y = x + b.
    The PSUM-source multiplies run on DVE; the rotate/combine tail runs
    on the otherwise-idle GpSimd so DVE can move to the next block."""
    half = RD // 2
    sl = slice(j * TQ, (j + 1) * TQ)
    A = mybir.AluOpType
    t1 = pool.tile([RD, TQ], F32, tag="rt1")
    nc.vector.scalar_tensor_tensor(t1[:], ps_ap, bias_ap, cos_s[:, sl],
                                   A.add, A.mult)
    t2 = pool.tile([RD, TQ], F32, tag="rt2")
    nc.vector.scalar_tensor_tensor(t2[:], ps_ap, bias_ap, sin_s[:, sl],
                                   A.add, A.mult)
    # rotate-half of t2 with sign baked in (single-input ops may shift
    # partitions; two-input SBUF ops must share the base partition)
    rot = pool.tile([RD, TQ], F32, tag="rrot")
    nc.vector.tensor_scalar_mul(rot[0:half, :], t2[half:RD, :], -1.0)
    nc.vector.tensor_copy(rot[half:RD, :], t2[0:half, :])
    nc.vector.tensor_add(out_ap, t1[:], rot[:])


# ---------------------------------------------------------------------------
# Host side: shard / preprocess / run / gather
# ---------------------------------------------------------------------------

_cached = {}


def _get_nc(loop=1):
    if loop not in _cached:
        _cached[loop] = build_nc(loop)
    return _cached[loop]


def _prep_inputs(x, Wq, bq, Wkv, bkv, Wkr, bkr, Wku, bku, Wvu, bvu, Wo, bo):
    """Build the 8 per-core input maps."""
    scale = 1.0 / np.sqrt(HD)
    bf = ml_dtypes.bfloat16

    pos = np.arange(T, dtype=np.float64)
    inv_freq = 1.0 / (THETA ** (np.arange(0, RD, 2, dtype=np.float64) / RD))
    ang = pos[:, None] * inv_freq            # (T, 32)
    cosT = np.concatenate([np.cos(ang), np.cos(ang)], -1).T.astype(np.float32)
    sinT = np.concatenate([np.sin(ang), np.sin(ang)], -1).T.astype(np.float32)
    cosT = np.ascontiguousarray(cosT)
    sinT = np.ascontiguousarray(sinT)

    # additive causal mask for the triangular corner of a diagonal tile
    r = np.arange(TKT)[:, None]
    c = np.arange(TKT)[None, :]
    maskneg = np.ascontiguousarray(
        np.where(c >= r, 0.0, NEG).astype(np.float32))

    ones_tk = np.ones((TKT, 1), dtype=bf)

    wkrT = np.ascontiguousarray(Wkr.T.astype(bf))

    # fold the latent down-projection into the per-head up-projections
    Wkv_k = Wkv[:LAT, :].astype(np.float32)      # (LAT, D)
    Wkv_v = Wkv[LAT:, :].astype(np.float32)
    bkv_k = bkv[:LAT].astype(np.float32)
    bkv_v = bkv[LAT:].astype(np.float32)
    Wk_eff = Wku.astype(np.float32) @ Wkv_k      # (H*CD, D)
    Wv_eff = Wvu.astype(np.float32) @ Wkv_v      # (H*HD, D)
    bk_eff = bku.astype(np.float32) + Wku.astype(np.float32) @ bkv_k
    bvu_eff = bvu.astype(np.float32) + Wvu.astype(np.float32) @ bkv_v

    in_maps = []
    for core in range(NCORES):
        b = core // 4
        hg = core % 4
        he = slice(hg * NH * HD, (hg + 1) * NH * HD)      # 512 q/v dims
        hc = slice(hg * NH * CD, (hg + 1) * NH * CD)      # 256 k-content dims
        biases = np.zeros((128, 21), dtype=np.float32)
        bqh = (bq[he] * scale).reshape(4, 128).T        # [128, head]
        biases[:, 0:4] = bqh
        biases[0:RD, 12] = bkr
        biases[0:RD, 13:17] = bqh[CD:, :]               # rope-row biases
        biases[0:CD, 17:21] = bk_eff[hc].reshape(4, CD).T  # per-head k biases
        in_maps.append({
            "xT": np.ascontiguousarray(x[b].T.astype(bf)),
            "wqT": np.ascontiguousarray((Wq[he, :] * scale).T.astype(bf)),
            "wkT": np.ascontiguousarray(Wk_eff[hc, :].T.astype(bf)),
            "wvT": np.ascontiguousarray(Wv_eff[he, :].T.astype(bf)),
            "wkrT": wkrT,
            "biases": np.ascontiguousarray(biases),
            "woT": np.ascontiguousarray(Wo[:, he].T.astype(bf)),
            "cosT": cosT,
            "sinT": sinT,
            "maskneg": maskneg,
            "ones_tk": ones_tk,
        })
    return in_maps


def kernel(**inputs):
    inputs = {k: np.asarray(v) for k, v in inputs.items()}
    in_maps = _prep_inputs(**inputs)
    nc = _get_nc(loop=1)
    res = run_bass_kernel_spmd(nc, in_maps, core_ids=list(range(NCORES)))

    Wo, Wvu, Wkv, bkv, bvu, bo = (inputs["Wo"], inputs["Wvu"], inputs["Wkv"],
                                  inputs["bkv"], inputs["bvu"], inputs["bo"])
    bvu_eff = (bvu.astype(np.float64)
               + Wvu.astype(np.float64) @ bkv[LAT:].astype(np.float64))
    const = (Wo.astype(np.float64) @ bvu_eff
             + bo.astype(np.float64)).astype(np.float32)

    out = np.zeros((B, T, D), dtype=np.float32)
    for core in range(NCORES):
        b = core // 4
        out[b] += res.results[core]["yT"].T.astype(np.float32)
    out += const[None, None, :]
    return out


# revision 21
# speedup vs baseline: 1.1404x; 1.1404x over previous
"""Multi-Head Latent Attention (MLA) Trainium2 kernel.

Problem: B=2, T=2048, D=2048, H=16 heads, HD=128, LAT=512, RD=64, CD=64.
Sharding: 8 cores = (batch 2) x (head-group 4). Each core handles one
batch and 4 heads: q/k/v/out projections sharded by head.

The latent kv down-projection is never materialized on device: the
per-head k-content and v projections are folded with the down-
projection on the host (Wk_eff = Wku[heads] @ Wkv_k, Wv_eff =
Wvu[heads] @ Wkv_v), so each core computes its heads' K/V directly
from x.  Per core that is 196.5k PE cycles instead of 311k for the
replicated-latent scheme, and needs no cross-core collective (a 4-core
AllGather of the latent measures ~190us here, far too slow to hide).

Engine balance (cost-model profile): PE ~263us is the critical
resource; exp lives on Act, bias-add/copies on Act, P-sum denominator
on DVE (bf16, 2x mode) + GpSimd partition_all_reduce, out-proj
PSUM->SBUF copies on GpSimd.  All DRAM tensors are host-packed into
SBUF-tile layout so every load/store is one large DMA (per-DMA issue
cost ~1.2us); inputs stream on the sync queue, weights on the GpSimd
SWDGE queue.

Per-core data layouts (all "T" suffixed = transposed, feature-major):
  xT      packed [8][128, 2*T]  bf16  (pairs of 128-row d-tiles of x[b].T)
  qT_s    per head [HD=128, T]     (scale 1/sqrt(HD) folded into Wq, RoPE applied)
  kT_s    per head [HD=128, T]     rows 0:64 content (direct), 64:128 shared rope key
  v_s     per tk-tile [128, 4*HD=512]  v in natural [t, e] layout (direct)
  scores  S^T [tk, tq] in PSUM  ->  exp -> P^T bf16 in SBUF
  pa      bf16 running sum of P^T tiles (DVE 2x) -> gpsimd partition_all_reduce
  outT    [HD, T] accumulated in PSUM, normalized by softmax denom
  yT      packed [4][128, 16*TQ] bf16 partial output (host adds Wo@bvu_eff + bo)
"""

import sys
import numpy as np
import ml_dtypes

sys.path.insert(0, "/opt/trn_rl_repo")

import concourse.bass as bass
import concourse.bass_isa as bass_isa
import concourse.bacc as bacc
import concourse.tile as tile
import concourse.mybir as mybir
from concourse.bass_utils import run_bass_kernel_spmd

B, T, D = 2, 2048, 2048
H, HD, LAT, RD = 16, 128, 512, 64
CD = HD - RD
THETA = 10000.0
NH = 4            # heads per core
NCORES = 8
TQ = 512          # tq block (matmul moving free dim)
TKT = 128         # tk tile (stationary)
NEG = -10000.0    # additive causal mask

BF = mybir.dt.bfloat16
F32 = mybir.dt.float32

ND = D // 128          # 16 d-tiles
NJ = T // TQ           # 4 tq blocks
NKT = T // TKT         # 16 tk tiles
NXC = ND // 2          # 8 two-tile xT chunks


def build_nc(loop=1, serialize=False):
    nc = bacc.Bacc("TRN2", target_bir_lowering=False, debug=False)

    xT_d = nc.declare_dram_parameter("xT", [NXC, 128, 2 * T], BF, isOutput=False)
    wqT_d = nc.declare_dram_parameter("wqT", [128, ND * NH * HD], BF, isOutput=False)
    wkT_d = nc.declare_dram_parameter("wkT", [128, ND * NH * CD], BF, isOutput=False)
    wvT_d = nc.declare_dram_parameter("wvT", [128, ND * NH * HD], BF, isOutput=False)
    wkrT_d = nc.declare_dram_parameter("wkrT", [128, ND * RD], BF, isOutput=False)
    # packed per-partition biases, every used slice starting at partition 0:
    # cols 0..3 bq(head), 12 bkr, 13..16 bq-rope(head), 17..20 bk_eff(head)
    bias_d = nc.declare_dram_parameter("biases", [128, 21], F32, isOutput=False)
    woT_d = nc.declare_dram_parameter("woT", [128, NH * D], BF, isOutput=False)
    cos_d = nc.declare_dram_parameter("cosT", [RD, T], F32, isOutput=False)
    sin_d = nc.declare_dram_parameter("sinT", [RD, T], F32, isOutput=False)
    mask_d = nc.declare_dram_parameter("maskneg", [TKT, TKT], F32, isOutput=False)
    yT_d = nc.declare_dram_parameter("yT", [NJ, 128, NKT * TQ], BF, isOutput=True)

    serialize_ = serialize
    with tile.TileContext(nc) as tc:
        body(nc, tc, loop, locals())
    nc.compile()
    return nc


def body(nc, tc, loop, dr):
    serialize = dr["serialize_"]
    xT_d, wqT_d, bias_d = dr["xT_d"], dr["wqT_d"], dr["bias_d"]
    wkT_d, wvT_d, wkrT_d = dr["wkT_d"], dr["wvT_d"], dr["wkrT_d"]
    woT_d = dr["woT_d"]
    cos_d, sin_d, mask_d, yT_d = (
        dr["cos_d"], dr["sin_d"], dr["mask_d"], dr["yT_d"])
    AExp = mybir.ActivationFunctionType.Exp
    ACopy = mybir.ActivationFunctionType.Copy
    AIdent = mybir.ActivationFunctionType.Identity

    from contextlib import ExitStack

    with ExitStack() as ctx:
        # ---- persistent pools (live across phases) ----
        p_per = ctx.enter_context(tc.tile_pool(name="per", bufs=1))
        p_psum = ctx.enter_context(tc.tile_pool(name="psum", bufs=8, space="PSUM"))

        # persistent SBUF tensors
        qT_s = [p_per.tile([128, T], BF, name=f"qT{h}", tag=f"qT{h}") for h in range(NH)]
        kT_s = [p_per.tile([128, T], BF, name=f"kT{h}", tag=f"kT{h}") for h in range(NH)]
        v_s = [p_per.tile([128, NH * HD], BF, name=f"v{m}", tag=f"v{m}") for m in range(NKT)]
        krT_s = p_per.tile([RD, T], BF, tag="krT")
        mask_s = p_per.tile([TKT, TKT], F32, tag="mask")
        bias_s = p_per.tile([128, 21], F32, tag="bias")
        bq_s = [bias_s[:, i:i + 1] for i in range(NH)]
        bkr_s = bias_s[0:RD, 12:13]
        bqr_s = [bias_s[0:RD, 13 + h:14 + h] for h in range(NH)]
        bk_s = [bias_s[0:CD, 17 + h:18 + h] for h in range(NH)]

        for it_ in range(loop):
            with tc.tile_pool(name="ph1", bufs=1) as p_x:
                xT2_s = [p_x.tile([128, 2 * T], BF, name=f"xc{c}", tag=f"xc{c}")
                         for c in range(NXC)]

                def xsl(d, a, b):
                    off = (d % 2) * T
                    return xT2_s[d // 2][:, off + a:off + b]

                cos_s = p_x.tile([RD, T], F32, tag="cos")
                sin_s = p_x.tile([RD, T], F32, tag="sin")
                wqT_s = p_x.tile([128, ND * NH * HD], BF, tag="wqT")
                wq = lambda d, h: wqT_s[:, d * NH * HD + h * HD:
                                        d * NH * HD + (h + 1) * HD]

                wvT_s = p_x.tile([128, ND * NH * HD], BF, tag="wvT")
                wv = lambda d: wvT_s[:, d * NH * HD:(d + 1) * NH * HD]
                with tc.tile_pool(name="w1a", bufs=1) as p_w:
                    wkT_s = p_w.tile([128, ND * NH * CD], BF, tag="wkT")
                    wk = lambda d, p: wkT_s[:, d * NH * CD + p * 128:
                                            d * NH * CD + (p + 1) * 128]
                    wkrT_s = p_w.tile([128, ND * RD], BF, tag="wkrT")
                    wkr = lambda d: wkrT_s[:, d * RD:(d + 1) * RD]

                    # weights stream on the GpSimd SWDGE queue; x chunks
                    # on the sync queue; each is one large packed DMA.
                    # (wk head-chunk first so the very first matmul can
                    # start ~1.5us in rather than after the full 1MB.)
                    c4 = 4 * NH * CD
                    nc.gpsimd.dma_start(wkT_s[:, 0:c4], wkT_d[:, 0:c4])
                    nc.gpsimd.dma_start(wkrT_s[:], wkrT_d[:, :])
                    nc.gpsimd.dma_start(wkT_s[:, c4:], wkT_d[:, c4:])
                    for c in range(NXC):
                        nc.sync.dma_start(xT2_s[c][:], xT_d[c])
                        if c == 0 and serialize and it_ > 0:
                            # bench-only: gate this iteration's first
                            # consumed tile on the previous iteration's
                            # final output block so loop iterations
                            # measure fully serialized (x + 0*tok == x).
                            tok = p_x.tile([128, 4], BF, name="tok",
                                           tag="tok")
                            nc.sync.dma_start(
                                tok[:],
                                yT_d[NJ - 1, 0:128, NKT * TQ - 4:NKT * TQ])
                            nc.vector.scalar_tensor_tensor(
                                xT2_s[0][:, 0:4], tok[:], 0.0,
                                xT2_s[0][:, 0:4],
                                mybir.AluOpType.mult, mybir.AluOpType.add)
                    nc.gpsimd.dma_start(bias_s[:], bias_d[:, :])
                    nc.gpsimd.dma_start(cos_s[:], cos_d[:, :])
                    nc.gpsimd.dma_start(sin_s[:], sin_d[:, :])
                    nc.gpsimd.dma_start(wvT_s[:], wvT_d[:, :])
                    nc.gpsimd.dma_start(mask_s[:], mask_d[:, :])
                    nc.gpsimd.dma_start(wqT_s[:], wqT_d[:, :])

                    with tc.tile_pool(name="krtmp", bufs=3) as p_kr:
                        # joint ramp loop: k-content pair 0 + rope key use
                        # all 8 PSUM banks so the PE keeps pace with the
                        # xT DMA arrival rate (8 matmuls per d-tile).
                        pss0 = [p_psum.tile([128, TQ], F32,
                                            name=f"pkc0_{_j}", tag="mm")
                                for _j in range(NJ)]
                        pssr = [p_psum.tile([RD, TQ], F32,
                                            name=f"pkr{_j}", tag="mm")
                                for _j in range(NJ)]
                        for d in range(ND):
                            for j in range(NJ):
                                nc.tensor.matmul(
                                    pss0[j][:], wk(d, 0),
                                    xsl(d, j * TQ, (j + 1) * TQ),
                                    start=(d == 0), stop=(d == ND - 1))
                                nc.tensor.matmul(
                                    pssr[j][:], wkr(d),
                                    xsl(d, j * TQ, (j + 1) * TQ),
                                    start=(d == 0), stop=(d == ND - 1))
                        for j in range(NJ):
                            for hh in range(2):
                                nc.scalar.activation(
                                    kT_s[hh][0:CD, j * TQ:(j + 1) * TQ],
                                    pss0[j][hh * CD:(hh + 1) * CD, :],
                                    AIdent, bias=bk_s[hh])
                            _rope(nc, p_kr, pssr[j][:], bkr_s, cos_s, sin_s, j,
                                  krT_s[:, j * TQ:(j + 1) * TQ])
                        # k content pair 1
                        pss1 = [p_psum.tile([128, TQ], F32,
                                            name=f"pkc1_{_j}", tag="mm")
                                for _j in range(NJ)]
                        for d in range(ND):
                            for j in range(NJ):
                                nc.tensor.matmul(
                                    pss1[j][:], wk(d, 1),
                                    xsl(d, j * TQ, (j + 1) * TQ),
                                    start=(d == 0), stop=(d == ND - 1))
                        for j in range(NJ):
                            for hh in range(2):
                                nc.scalar.activation(
                                    kT_s[2 + hh][0:CD, j * TQ:(j + 1) * TQ],
                                    pss1[j][hh * CD:(hh + 1) * CD, :],
                                    AIdent, bias=bk_s[2 + hh])
                        # shared rope rows into each head's k (Act engine)
                        for h in range(NH):
                            nc.scalar.activation(kT_s[h][CD:HD, :], krT_s[:],
                                                 ACopy)

                # ---------- phase 1b: q projection (+ rope on last 64 dims) --
                if True:
                    with tc.tile_pool(name="qtmp", bufs=3) as p_qr:
                        for h in range(NH):
                            pss = [p_psum.tile([128, TQ], F32,
                                                name=f"pq{_j}", tag="mm")
                                   for _j in range(NJ)]
                            for d in range(ND):
                                for j in range(NJ):
                                    nc.tensor.matmul(
                                        pss[j][:], wq(d, h),
                                        xsl(d, j * TQ, (j + 1) * TQ),
                                        start=(d == 0), stop=(d == ND - 1))
                            for j in range(NJ):
                                # content rows 0:64 -> bias add on Act
                                nc.scalar.activation(
                                    qT_s[h][0:CD, j * TQ:(j + 1) * TQ],
                                    pss[j][0:CD, :],
                                    AIdent, bias=bq_s[h][0:CD, :])
                                # rope rows 64:128
                                _rope(nc, p_qr, pss[j][CD:HD, :],
                                      bqr_s[h],
                                      cos_s, sin_s, j,
                                      qT_s[h][CD:HD, j * TQ:(j + 1) * TQ])

                # v direct (last in phase 1 so its PE work covers the
                # q-head drain latency at the phase boundary):
                # [tk-tile, e] natural layout; stationary = xT column
                # tile, moving = wvT.
                for m in range(NKT):
                    ps = p_psum.tile([128, NH * HD], F32, tag="mm")
                    for d in range(ND):
                        nc.tensor.matmul(
                            ps[:],
                            xsl(d, m * 128, (m + 1) * 128),
                            wv(d),
                            start=(d == 0), stop=(d == ND - 1))
                    nc.scalar.activation(v_s[m][:], ps[:], ACopy)

            # ---------- phase 2: attention + out proj ----------
            with tc.tile_pool(name="ph2", bufs=1) as p_2:
                woT_s = p_2.tile([128, NH * D], BF, tag="woT")
                wo = lambda dl, eo: woT_s[:, dl * D + eo * 128:
                                          dl * D + (eo + 1) * 128]
                nc.sync.dma_start(woT_s[:], woT_d[:, :])
                aoT_s = [p_2.tile([128, T], BF, name=f"ao{h}", tag=f"ao{h}") for h in range(NH)]

                # ---------- attention (1-step software pipeline),
                # j-outer so each tq column's out-projection can be
                # interleaved as soon as all 4 heads of that column are
                # normalized; this fills PE bubbles and shrinks the tail.
                with tc.tile_pool(name="pT", bufs=1) as p_pT, \
                     tc.tile_pool(name="att", bufs=4) as p_att, \
                     tc.tile_pool(name="yout", bufs=2) as p_y:

                    def finish(h, j, pts, pa):
                        ntk = 4 * (j + 1)
                        # columns < lo(kk) of a diagonal tile are fully
                        # masked; skip them in every chain (tile kk=0 is
                        # always full-width, so start=True covers the bank)
                        lo = lambda kk: 128 * max(0, kk - 4 * j)
                        # denominator: partition all-reduce of the bf16
                        # P-tile running sum on the (idle) GpSimd engine;
                        # result lands broadcast on all 128 partitions.
                        den = p_att.tile([TKT, TQ], F32, tag="den")
                        nc.gpsimd.partition_all_reduce(
                            den[:], pa[:], channels=128,
                            reduce_op=bass_isa.ReduceOp.add)
                        # out^T accumulation
                        po = p_psum.tile([HD, TQ], F32, tag="mm")
                        for kk in range(ntk):
                            nc.tensor.matmul(
                                po[:, lo(kk):],
                                v_s[kk][:, h * HD:(h + 1) * HD],
                                pts[kk][:, lo(kk):],
                                start=(kk == 0), stop=(kk == ntk - 1))
                        rec = p_att.tile([TKT, TQ], F32, tag="rec")
                        nc.vector.reciprocal_approx_fast(rec[:], den[:])
                        nc.vector.tensor_mul(
                            aoT_s[h][:, j * TQ:(j + 1) * TQ], po[:], rec[:])

                    def out_proj_col(j):
                        ys = p_y.tile([128, NKT * TQ], BF, tag="y")
                        half = NKT * TQ // 2
                        for eo in range(D // 128):
                            ps = p_psum.tile([128, TQ], F32, tag="mm")
                            for dl in range(NH):
                                nc.tensor.matmul(
                                    ps[:], wo(dl, eo),
                                    aoT_s[dl][:, j * TQ:(j + 1) * TQ],
                                    start=(dl == 0), stop=(dl == NH - 1))
                            nc.vector.tensor_copy(
                                ys[:, eo * TQ:(eo + 1) * TQ], ps[:])
                            if j == NJ - 1 and eo == 7:
                                # shrink the kernel tail: first half of
                                # the last column streams out while the
                                # second half is still computing
                                nc.sync.dma_start(yT_d[j, :, 0:half],
                                                  ys[:, 0:half])
                        if j == NJ - 1:
                            nc.sync.dma_start(yT_d[j, :, half:],
                                              ys[:, half:])
                        else:
                            nc.sync.dma_start(yT_d[j], ys[:])

                    prev = None
                    for j in range(NJ):
                        for h in range(NH):
                            ntk = 4 * (j + 1)
                            pts = [p_pT.tile([TKT, TQ], BF, name=f"pT{kk}",
                                             tag=f"pT{kk}", bufs=2)
                                   for kk in range(ntk)]
                            pa = p_pT.tile([TKT, TQ], BF, name="paAcc",
                                           tag="paAcc", bufs=2)
                            for kk in range(ntk):
                                m = kk - 4 * j
                                lo = 128 * max(0, m)
                                ps = p_psum.tile([TKT, TQ], F32, tag="mm")
                                nc.tensor.matmul(
                                    ps[:, lo:],
                                    kT_s[h][:, kk * TKT:(kk + 1) * TKT],
                                    qT_s[h][:, j * TQ + lo:(j + 1) * TQ],
                                    start=True, stop=True)
                                if m >= 0:
                                    # triangular corner only; cols < lo are
                                    # skipped, cols >= lo+128 are all-pass
                                    nc.vector.tensor_add(
                                        ps[:, lo:lo + TKT], ps[:, lo:lo + TKT],
                                        mask_s[:])
                                nc.scalar.activation(
                                    pts[kk][:, lo:], ps[:, lo:], AExp)
                                # running bf16 sum of P tiles for the
                                # softmax denominator (masked cols excluded)
                                if kk == 0:
                                    nc.vector.tensor_copy(pa[:], pts[0][:])
                                else:
                                    nc.vector.tensor_add(
                                        pa[:, lo:], pa[:, lo:],
                                        pts[kk][:, lo:])
                            if prev is not None:
                                finish(*prev)
                                if prev[0] == NH - 1:
                                    out_proj_col(prev[1])
                            prev = (h, j, pts, pa)
                    finish(*prev)
                    out_proj_col(NJ - 1)


def _rope(nc, pool, ps_ap, bias_ap, cos_s, sin_s, j, out_ap):
    """RoPE on a [64, TQ] PSUM block (rotate-half, RD=64), bf16 out.
    out[0:32] = y[0:32]*cos[0:32] - y[32:64]*sin[0:32]
    out[32:64] = y[32:64]*cos[32:64] + y[0:32]*sin[32:64],  y = x + b."""
    half = RD // 2
    sl = slice(j * TQ, (j + 1) * TQ)
    A = mybir.AluOpType
    t1 = pool.tile([RD, TQ], BF, tag="rt1")
    nc.vector.scalar_tensor_tensor(t1[:], ps_ap, bias_ap, cos_s[:, sl],
                                   A.add, A.mult)
    t2 = pool.tile([RD, TQ], BF, tag="rt2")
    nc.vector.scalar_tensor_tensor(t2[:], ps_ap, bias_ap, sin_s[:, sl],
                                   A.add, A.mult)
    # rotate-half of t2 with sign baked in (single-input ops may shift
    # partitions; two-input SBUF ops must share the base partition);
    # bf16 intermediates get the 2x DVE perf mode.
    rot = pool.tile([RD, TQ], BF, tag="rrot")
    nc.vector.tensor_scalar_mul(rot[0:half, :], t2[half:RD, :], -1.0)
    nc.vector.tensor_copy(rot[half:RD, :], t2[0:half, :])
    nc.vector.tensor_add(out_ap, t1[:], rot[:])


# ---------------------------------------------------------------------------
# Host side: shard / preprocess / run / gather
# ---------------------------------------------------------------------------

_cached = {}


def _get_nc(loop=1):
    if loop not in _cached:
        _cached[loop] = build_nc(loop)
    return _cached[loop]


def _pack_dtiles(w, cols):
    """[D, cols] row-major -> [128, ND*cols] with d-tile i at col i*cols."""
    return np.ascontiguousarray(
        w.reshape(ND, 128, cols).transpose(1, 0, 2).reshape(128, ND * cols))


def _prep_inputs(x, Wq, bq, Wkv, bkv, Wkr, bkr, Wku, bku, Wvu, bvu, Wo, bo):
    """Build the 8 per-core input maps."""
    scale = 1.0 / np.sqrt(HD)
    bf = ml_dtypes.bfloat16

    pos = np.arange(T, dtype=np.float64)
    inv_freq = 1.0 / (THETA ** (np.arange(0, RD, 2, dtype=np.float64) / RD))
    ang = pos[:, None] * inv_freq            # (T, 32)
    cosT = np.concatenate([np.cos(ang), np.cos(ang)], -1).T.astype(np.float32)
    sinT = np.concatenate([np.sin(ang), np.sin(ang)], -1).T.astype(np.float32)
    cosT = np.ascontiguousarray(cosT)
    sinT = np.ascontiguousarray(sinT)

    # additive causal mask for the triangular corner of a diagonal tile
    r = np.arange(TKT)[:, None]
    c = np.arange(TKT)[None, :]
    maskneg = np.ascontiguousarray(
        np.where(c >= r, 0.0, NEG).astype(np.float32))

    wkrT = _pack_dtiles(Wkr.T.astype(bf), RD)

    # fold the latent down-projection into the per-head up-projections
    Wkv_k = Wkv[:LAT, :].astype(np.float32)      # (LAT, D)
    Wkv_v = Wkv[LAT:, :].astype(np.float32)
    bkv_k = bkv[:LAT].astype(np.float32)
    bkv_v = bkv[LAT:].astype(np.float32)
    Wk_eff = Wku.astype(np.float32) @ Wkv_k      # (H*CD, D)
    Wv_eff = Wvu.astype(np.float32) @ Wkv_v      # (H*HD, D)
    bk_eff = bku.astype(np.float32) + Wku.astype(np.float32) @ bkv_k

    in_maps = []
    for core in range(NCORES):
        b = core // 4
        hg = core % 4
        he = slice(hg * NH * HD, (hg + 1) * NH * HD)      # 512 q/v dims
        hc = slice(hg * NH * CD, (hg + 1) * NH * CD)      # 256 k-content dims
        biases = np.zeros((128, 21), dtype=np.float32)
        bqh = (bq[he] * scale).reshape(4, 128).T        # [128, head]
        biases[:, 0:4] = bqh
        biases[0:RD, 12] = bkr
        biases[0:RD, 13:17] = bqh[CD:, :]               # rope-row biases
        biases[0:CD, 17:21] = bk_eff[hc].reshape(4, CD).T  # per-head k biases
        xb = np.ascontiguousarray(
            x[b].T.astype(bf).reshape(NXC, 2, 128, T)
            .transpose(0, 2, 1, 3).reshape(NXC, 128, 2 * T))
        in_maps.append({
            "xT": xb,
            "wqT": _pack_dtiles((Wq[he, :] * scale).T.astype(bf), NH * HD),
            "wkT": _pack_dtiles(Wk_eff[hc, :].T.astype(bf), NH * CD),
            "wvT": _pack_dtiles(Wv_eff[he, :].T.astype(bf), NH * HD),
            "wkrT": wkrT,
            "biases": np.ascontiguousarray(biases),
            "woT": np.ascontiguousarray(
                Wo[:, he].T.astype(bf).reshape(NH, 128, D)
                .transpose(1, 0, 2).reshape(128, NH * D)),
            "cosT": cosT,
            "sinT": sinT,
            "maskneg": maskneg,
        })
    return in_maps


def kernel(**inputs):
    inputs = {k: np.asarray(v) for k, v in inputs.items()}
    in_maps = _prep_inputs(**inputs)
    nc = _get_nc(loop=1)
    res = run_bass_kernel_spmd(nc, in_maps, core_ids=list(range(NCORES)))

    Wo, Wvu, Wkv, bkv, bvu, bo = (inputs["Wo"], inputs["Wvu"], inputs["Wkv"],
                                  inputs["bkv"], inputs["bvu"], inputs["bo"])
    bvu_eff = (bvu.astype(np.float64)
               + Wvu.astype(np.float64) @ bkv[LAT:].astype(np.float64))
    const = (Wo.astype(np.float64) @ bvu_eff
             + bo.astype(np.float64)).astype(np.float32)

    out = np.zeros((B, T, D), dtype=np.float32)
    for core in range(NCORES):
        b = core // 4
        # yT packed [NJ, 128, NKT*TQ]: block eo at free offset eo*TQ of
        # column block j  ->  y[eo*128+p, j*TQ+c]
        yT = (res.results[core]["yT"].astype(np.float32)
              .reshape(NJ, 128, NKT, TQ).transpose(2, 1, 0, 3)
              .reshape(D, T))
        out[b] += yT.T
    out += const[None, None, :]
    return out


# revision 23
# speedup vs baseline: 1.4510x; 1.2724x over previous
"""Multi-Head Latent Attention (MLA) Trainium2 kernel.

Problem: B=2, T=2048, D=2048, H=16 heads, HD=128, LAT=512, RD=64, CD=64.
Sharding: 8 cores = (batch 2) x (head-group 4). Each core handles one
batch and 4 heads: q/k/v/out projections sharded by head.

The latent kv down-projection is never materialized on device: the
per-head k-content and v projections are folded with the down-
projection on the host (Wk_eff = Wku[heads] @ Wkv_k, Wv_eff =
Wvu[heads] @ Wkv_v), so each core computes its heads' K/V directly
from x.  Per core that is 196.5k PE cycles instead of 311k for the
replicated-latent scheme, and needs no cross-core collective (a 4-core
AllGather of the latent measures ~190us here, far too slow to hide).

Engine balance (cost-model profile): PE ~263us is the critical
resource; exp lives on Act, bias-add/copies on Act, P-sum denominator
on DVE (bf16, 2x mode) + GpSimd partition_all_reduce, out-proj
PSUM->SBUF copies on GpSimd.  All DRAM tensors are host-packed into
SBUF-tile layout so every load/store is one large DMA (per-DMA issue
cost ~1.2us); inputs stream on the sync queue, weights on the GpSimd
SWDGE queue.

Per-core data layouts (all "T" suffixed = transposed, feature-major):
  xT      packed [8][128, 2*T]  bf16  (pairs of 128-row d-tiles of x[b].T)
  qT_s    per head [HD=128, T]     (scale 1/sqrt(HD) folded into Wq, RoPE applied)
  kT_s    per head [HD=128, T]     rows 0:64 content (direct), 64:128 shared rope key
  v_s     per tk-tile [128, 4*HD=512]  v in natural [t, e] layout (direct)
  scores  S^T [tk, tq] in PSUM  ->  exp -> P^T bf16 in SBUF
  pa      bf16 running sum of P^T tiles (DVE 2x) -> gpsimd partition_all_reduce
  outT    [HD, T] accumulated in PSUM, normalized by softmax denom
  yT      packed [4][128, 16*TQ] bf16 partial output (host adds Wo@bvu_eff + bo)
"""

import sys
import numpy as np
import ml_dtypes

sys.path.insert(0, "/opt/trn_rl_repo")

import concourse.bass as bass
import concourse.bass_isa as bass_isa
import concourse.bacc as bacc
import concourse.tile as tile
import concourse.mybir as mybir
from concourse.bass_utils import run_bass_kernel_spmd

B, T, D = 2, 2048, 2048
H, HD, LAT, RD = 16, 128, 512, 64
CD = HD - RD
THETA = 10000.0
NH = 4            # heads per core
NCORES = 8
TQ = 512          # tq block (matmul moving free dim)
TKT = 128         # tk tile (stationary)
NEG = -10000.0    # additive causal mask

BF = mybir.dt.bfloat16
F32 = mybir.dt.float32

ND = D // 128          # 16 d-tiles
NJ = T // TQ           # 4 tq blocks
NKT = T // TKT         # 16 tk tiles
NXC = ND // 2          # 8 two-tile xT chunks


def build_nc(loop=1, serialize=False):
    nc = bacc.Bacc("TRN2", target_bir_lowering=False, debug=False)

    xT_d = nc.declare_dram_parameter("xT", [NXC, 128, 2 * T], BF, isOutput=False)
    wqT_d = nc.declare_dram_parameter("wqT", [128, ND * NH * HD], BF, isOutput=False)
    wkT_d = nc.declare_dram_parameter("wkT", [128, ND * NH * CD], BF, isOutput=False)
    wvT_d = nc.declare_dram_parameter("wvT", [128, ND * NH * HD], BF, isOutput=False)
    wkrT_d = nc.declare_dram_parameter("wkrT", [128, ND * RD], BF, isOutput=False)
    # packed per-partition biases, every used slice starting at partition 0:
    # cols 0..3 bq(head), 12 bkr, 13..16 bq-rope(head), 17..20 bk_eff(head)
    bias_d = nc.declare_dram_parameter("biases", [128, 21], F32, isOutput=False)
    woT_d = nc.declare_dram_parameter("woT", [128, NH * D], BF, isOutput=False)
    cos_d = nc.declare_dram_parameter("cosT", [RD, T], F32, isOutput=False)
    sin_d = nc.declare_dram_parameter("sinT", [RD, T], F32, isOutput=False)
    mask_d = nc.declare_dram_parameter("maskneg", [TKT, TKT], F32, isOutput=False)
    yT_d = nc.declare_dram_parameter("yT", [NJ, 128, NKT * TQ], BF, isOutput=True)

    serialize_ = serialize
    with tile.TileContext(nc) as tc:
        body(nc, tc, loop, locals())
    nc.compile()
    return nc


def body(nc, tc, loop, dr):
    serialize = dr["serialize_"]
    xT_d, wqT_d, bias_d = dr["xT_d"], dr["wqT_d"], dr["bias_d"]
    wkT_d, wvT_d, wkrT_d = dr["wkT_d"], dr["wvT_d"], dr["wkrT_d"]
    woT_d = dr["woT_d"]
    cos_d, sin_d, mask_d, yT_d = (
        dr["cos_d"], dr["sin_d"], dr["mask_d"], dr["yT_d"])
    AExp = mybir.ActivationFunctionType.Exp
    ACopy = mybir.ActivationFunctionType.Copy
    AIdent = mybir.ActivationFunctionType.Identity

    from contextlib import ExitStack

    with ExitStack() as ctx:
        # ---- persistent pools (live across phases) ----
        p_per = ctx.enter_context(tc.tile_pool(name="per", bufs=1))

        # persistent SBUF tensors
        qT_s = [p_per.tile([128, T], BF, name=f"qT{h}", tag=f"qT{h}") for h in range(NH)]
        kT_s = [p_per.tile([128, T], BF, name=f"kT{h}", tag=f"kT{h}") for h in range(NH)]
        v_s = [p_per.tile([128, NH * HD], BF, name=f"v{m}", tag=f"v{m}") for m in range(NKT)]
        krT_s = p_per.tile([RD, T], BF, tag="krT")
        mask_s = p_per.tile([TKT, TKT], F32, tag="mask")
        bias_s = p_per.tile([128, 21], F32, tag="bias")
        bq_s = [bias_s[:, i:i + 1] for i in range(NH)]
        bkr_s = bias_s[0:RD, 12:13]
        bqr_s = [bias_s[0:RD, 13 + h:14 + h] for h in range(NH)]
        bk_s = [bias_s[0:CD, 17 + h:18 + h] for h in range(NH)]

        for it_ in range(loop):
            with tc.tile_pool(name="ph1", bufs=1) as p_x, \
                 tc.tile_pool(name="psum1", bufs=8, space="PSUM") as p_psum:
                xT2_s = [p_x.tile([128, 2 * T], BF, name=f"xc{c}", tag=f"xc{c}")
                         for c in range(NXC)]

                def xsl(d, a, b):
                    off = (d % 2) * T
                    return xT2_s[d // 2][:, off + a:off + b]

                cos_s = p_x.tile([RD, T], F32, tag="cos")
                sin_s = p_x.tile([RD, T], F32, tag="sin")
                wqT_s = p_x.tile([128, ND * NH * HD], BF, tag="wqT")
                wq = lambda d, h: wqT_s[:, d * NH * HD + h * HD:
                                        d * NH * HD + (h + 1) * HD]

                wvT_s = p_x.tile([128, ND * NH * HD], BF, tag="wvT")
                wv = lambda d: wvT_s[:, d * NH * HD:(d + 1) * NH * HD]
                with tc.tile_pool(name="w1a", bufs=1) as p_w:
                    wkT_s = p_w.tile([128, ND * NH * CD], BF, tag="wkT")
                    wk = lambda d, p: wkT_s[:, d * NH * CD + p * 128:
                                            d * NH * CD + (p + 1) * 128]
                    wkrT_s = p_w.tile([128, ND * RD], BF, tag="wkrT")
                    wkr = lambda d: wkrT_s[:, d * RD:(d + 1) * RD]

                    # weights stream on the GpSimd SWDGE queue; x chunks
                    # on the sync queue; each is one large packed DMA.
                    # (wk head-chunk first so the very first matmul can
                    # start ~1.5us in rather than after the full 1MB.)
                    c4 = 4 * NH * CD
                    nc.gpsimd.dma_start(wkT_s[:, 0:c4], wkT_d[:, 0:c4])
                    nc.gpsimd.dma_start(wkrT_s[:], wkrT_d[:, :])
                    nc.gpsimd.dma_start(wkT_s[:, c4:], wkT_d[:, c4:])
                    for c in range(NXC):
                        nc.sync.dma_start(xT2_s[c][:], xT_d[c])
                        if c == 0 and serialize and it_ > 0:
                            # bench-only: gate this iteration's first
                            # consumed tile on the previous iteration's
                            # final output block so loop iterations
                            # measure fully serialized (x + 0*tok == x).
                            tok = p_x.tile([128, 4], BF, name="tok",
                                           tag="tok")
                            nc.sync.dma_start(
                                tok[:],
                                yT_d[NJ - 1, 0:128, NKT * TQ - 4:NKT * TQ])
                            nc.vector.scalar_tensor_tensor(
                                xT2_s[0][:, 0:4], tok[:], 0.0,
                                xT2_s[0][:, 0:4],
                                mybir.AluOpType.mult, mybir.AluOpType.add)
                    nc.gpsimd.dma_start(bias_s[:], bias_d[:, :])
                    nc.gpsimd.dma_start(cos_s[:], cos_d[:, :])
                    nc.gpsimd.dma_start(sin_s[:], sin_d[:, :])
                    nc.gpsimd.dma_start(wvT_s[:], wvT_d[:, :])
                    nc.gpsimd.dma_start(mask_s[:], mask_d[:, :])
                    nc.gpsimd.dma_start(wqT_s[:], wqT_d[:, :])

                    with tc.tile_pool(name="krtmp", bufs=3) as p_kr:
                        # joint ramp loop: k-content pair 0 + rope key use
                        # all 8 PSUM banks so the PE keeps pace with the
                        # xT DMA arrival rate (8 matmuls per d-tile).
                        pss0 = [p_psum.tile([128, TQ], F32,
                                            name=f"pkc0_{_j}", tag="mm")
                                for _j in range(NJ)]
                        pssr = [p_psum.tile([RD, TQ], F32,
                                            name=f"pkr{_j}", tag="mm")
                                for _j in range(NJ)]
                        for d in range(ND):
                            for j in range(NJ):
                                nc.tensor.matmul(
                                    pss0[j][:], wk(d, 0),
                                    xsl(d, j * TQ, (j + 1) * TQ),
                                    start=(d == 0), stop=(d == ND - 1))
                                nc.tensor.matmul(
                                    pssr[j][:], wkr(d),
                                    xsl(d, j * TQ, (j + 1) * TQ),
                                    start=(d == 0), stop=(d == ND - 1))
                        for j in range(NJ):
                            for hh in range(2):
                                nc.scalar.activation(
                                    kT_s[hh][0:CD, j * TQ:(j + 1) * TQ],
                                    pss0[j][hh * CD:(hh + 1) * CD, :],
                                    AIdent, bias=bk_s[hh])
                            _rope(nc, p_kr, pssr[j][:], bkr_s, cos_s, sin_s, j,
                                  krT_s[:, j * TQ:(j + 1) * TQ])
                        # k content pair 1
                        pss1 = [p_psum.tile([128, TQ], F32,
                                            name=f"pkc1_{_j}", tag="mm")
                                for _j in range(NJ)]
                        for d in range(ND):
                            for j in range(NJ):
                                nc.tensor.matmul(
                                    pss1[j][:], wk(d, 1),
                                    xsl(d, j * TQ, (j + 1) * TQ),
                                    start=(d == 0), stop=(d == ND - 1))
                        for j in range(NJ):
                            for hh in range(2):
                                nc.scalar.activation(
                                    kT_s[2 + hh][0:CD, j * TQ:(j + 1) * TQ],
                                    pss1[j][hh * CD:(hh + 1) * CD, :],
                                    AIdent, bias=bk_s[2 + hh])
                        # shared rope rows into each head's k (Act engine)
                        for h in range(NH):
                            nc.scalar.activation(kT_s[h][CD:HD, :], krT_s[:],
                                                 ACopy)

                # ---------- phase 1b: q projection (+ rope on last 64 dims) --
                if True:
                    with tc.tile_pool(name="qtmp", bufs=3) as p_qr:
                        for h in range(NH):
                            pss = [p_psum.tile([128, TQ], F32,
                                                name=f"pq{_j}", tag="mm")
                                   for _j in range(NJ)]
                            for d in range(ND):
                                for j in range(NJ):
                                    nc.tensor.matmul(
                                        pss[j][:], wq(d, h),
                                        xsl(d, j * TQ, (j + 1) * TQ),
                                        start=(d == 0), stop=(d == ND - 1))
                            for j in range(NJ):
                                # content rows 0:64 -> bias add on Act
                                nc.scalar.activation(
                                    qT_s[h][0:CD, j * TQ:(j + 1) * TQ],
                                    pss[j][0:CD, :],
                                    AIdent, bias=bq_s[h][0:CD, :])
                                # rope rows 64:128
                                _rope(nc, p_qr, pss[j][CD:HD, :],
                                      bqr_s[h],
                                      cos_s, sin_s, j,
                                      qT_s[h][CD:HD, j * TQ:(j + 1) * TQ])

                # v direct (last in phase 1 so its PE work covers the
                # q-head drain latency at the phase boundary):
                # [tk-tile, e] natural layout; stationary = xT column
                # tile, moving = wvT.
                for m in range(NKT):
                    ps = p_psum.tile([128, NH * HD], F32, tag="mm")
                    for d in range(ND):
                        nc.tensor.matmul(
                            ps[:],
                            xsl(d, m * 128, (m + 1) * 128),
                            wv(d),
                            start=(d == 0), stop=(d == ND - 1))
                    nc.scalar.activation(v_s[m][:], ps[:], ACopy)

            # ---------- phase 2: attention + out proj ----------
            with tc.tile_pool(name="ph2", bufs=1) as p_2:
                woT_s = p_2.tile([128, NH * D], BF, tag="woT")
                wo = lambda dl, eo: woT_s[:, dl * D + eo * 128:
                                          dl * D + (eo + 1) * 128]
                nc.sync.dma_start(woT_s[:], woT_d[:, :])
                aoT_s = [p_2.tile([128, T], BF, name=f"ao{h}", tag=f"ao{h}") for h in range(NH)]

                # ---------- attention (1-step software pipeline),
                # j-outer so each tq column's out-projection can be
                # interleaved as soon as all 4 heads of that column are
                # normalized; this fills PE bubbles and shrinks the tail.
                with tc.tile_pool(name="pT", bufs=1) as p_pT, \
                     tc.tile_pool(name="att", bufs=4) as p_att, \
                     tc.tile_pool(name="psum2", bufs=4, space="PSUM") as p_mm, \
                     tc.tile_pool(name="psumS", bufs=2, space="PSUM") as p_s2, \
                     tc.tile_pool(name="yout", bufs=2) as p_y:

                    def finish(h, j, pts, pa):
                        ntk = 4 * (j + 1)
                        # columns < lo(kk) of a diagonal tile are fully
                        # masked; skip them in every chain (tile kk=0 is
                        # always full-width, so start=True covers the bank)
                        lo = lambda kk: 128 * max(0, kk - 4 * j)
                        # denominator: partition all-reduce of the bf16
                        # P-tile running sum on the (idle) GpSimd engine;
                        # result lands broadcast on all 128 partitions.
                        den = p_att.tile([TKT, TQ], F32, tag="den")
                        nc.gpsimd.partition_all_reduce(
                            den[:], pa[:], channels=128,
                            reduce_op=bass_isa.ReduceOp.add)
                        # out^T accumulation
                        po = p_mm.tile([HD, TQ], F32, tag="mm")
                        for kk in range(ntk):
                            pt, base = pts[kk]
                            nc.tensor.matmul(
                                po[:, lo(kk):],
                                v_s[kk][:, h * HD:(h + 1) * HD],
                                pt[:, base + lo(kk):base + TQ],
                                start=(kk == 0), stop=(kk == ntk - 1))
                        rec = p_att.tile([TKT, TQ], F32, tag="rec")
                        nc.vector.reciprocal_approx_fast(rec[:], den[:])
                        nc.vector.tensor_mul(
                            aoT_s[h][:, j * TQ:(j + 1) * TQ], po[:], rec[:])

                    def out_proj_col(j):
                        ys = p_y.tile([128, NKT * TQ], BF, tag="y")
                        half = NKT * TQ // 2
                        for eo in range(D // 128):
                            ps = p_mm.tile([128, TQ], F32, tag="mm")
                            for dl in range(NH):
                                nc.tensor.matmul(
                                    ps[:], wo(dl, eo),
                                    aoT_s[dl][:, j * TQ:(j + 1) * TQ],
                                    start=(dl == 0), stop=(dl == NH - 1))
                            nc.vector.tensor_copy(
                                ys[:, eo * TQ:(eo + 1) * TQ], ps[:])
                            if j == NJ - 1 and eo == 7:
                                # shrink the kernel tail: first half of
                                # the last column streams out while the
                                # second half is still computing
                                nc.sync.dma_start(yT_d[j, :, 0:half],
                                                  ys[:, 0:half])
                        if j == NJ - 1:
                            nc.sync.dma_start(yT_d[j, :, half:],
                                              ys[:, half:])
                        else:
                            nc.sync.dma_start(yT_d[j], ys[:])

                    prev = None
                    for j in range(NJ):
                        npair = (4 * j) // 2     # non-diagonal tile pairs
                        for h in range(NH):
                            pts = []             # (tile, base_col) per kk
                            pa = p_pT.tile([TKT, TQ], BF, name="paAcc",
                                           tag="paAcc", bufs=2)
                            # non-diagonal tiles two at a time: one exp
                            # (and one PSUM drain) per 1024 columns.
                            for pp in range(npair):
                                ps2 = p_s2.tile([TKT, 2 * TQ], F32,
                                                tag="ps2")
                                for s in range(2):
                                    kk = 2 * pp + s
                                    nc.tensor.matmul(
                                        ps2[:, s * TQ:(s + 1) * TQ],
                                        kT_s[h][:, kk * TKT:(kk + 1) * TKT],
                                        qT_s[h][:, j * TQ:(j + 1) * TQ],
                                        start=True, stop=True)
                                pt2 = p_pT.tile([TKT, 2 * TQ], BF,
                                                name=f"pT2_{pp}",
                                                tag=f"pT2_{pp}", bufs=2)
                                nc.scalar.activation(pt2[:], ps2[:], AExp)
                                if pp == 0:
                                    nc.vector.tensor_copy(pa[:],
                                                          pt2[:, 0:TQ])
                                else:
                                    nc.vector.tensor_add(
                                        pa[:], pa[:], pt2[:, 0:TQ])
                                nc.vector.tensor_add(
                                    pa[:], pa[:], pt2[:, TQ:2 * TQ])
                                pts += [(pt2, 0), (pt2, TQ)]
                            # finish the previous block between the
                            # (Act-heavy) paired production and the diag
                            # tiles: the AV chain gives the PE work while
                            # the exp backlog drains.
                            if prev is not None and npair > 0:
                                finish(*prev)
                                if prev[0] == NH - 1:
                                    out_proj_col(prev[1])
                                prev = None
                            # diagonal tiles, triangular corner masked
                            for m in range(4):
                                kk = 4 * j + m
                                lo = 128 * m
                                ps = p_mm.tile([TKT, TQ], F32, tag="mm")
                                nc.tensor.matmul(
                                    ps[:, lo:],
                                    kT_s[h][:, kk * TKT:(kk + 1) * TKT],
                                    qT_s[h][:, j * TQ + lo:(j + 1) * TQ],
                                    start=True, stop=True)
                                nc.vector.tensor_add(
                                    ps[:, lo:lo + TKT], ps[:, lo:lo + TKT],
                                    mask_s[:])
                                ptd = p_pT.tile([TKT, TQ], BF,
                                                name=f"pTd{m}",
                                                tag=f"pTd{m}", bufs=2)
                                nc.scalar.activation(
                                    ptd[:, lo:], ps[:, lo:], AExp)
                                if kk == 0:
                                    nc.vector.tensor_copy(pa[:], ptd[:])
                                else:
                                    nc.vector.tensor_add(
                                        pa[:, lo:], pa[:, lo:],
                                        ptd[:, lo:])
                                pts.append((ptd, 0))
                            if prev is not None:
                                finish(*prev)
                                if prev[0] == NH - 1:
                                    out_proj_col(prev[1])
                            prev = (h, j, pts, pa)
                    finish(*prev)
                    out_proj_col(NJ - 1)


def _rope(nc, pool, ps_ap, bias_ap, cos_s, sin_s, j, out_ap):
    """RoPE on a [64, TQ] PSUM block (rotate-half, RD=64), bf16 out.
    out[0:32] = y[0:32]*cos[0:32] - y[32:64]*sin[0:32]
    out[32:64] = y[32:64]*cos[32:64] + y[0:32]*sin[32:64],  y = x + b."""
    half = RD // 2
    sl = slice(j * TQ, (j + 1) * TQ)
    A = mybir.AluOpType
    t1 = pool.tile([RD, TQ], BF, tag="rt1")
    nc.vector.scalar_tensor_tensor(t1[:], ps_ap, bias_ap, cos_s[:, sl],
                                   A.add, A.mult)
    t2 = pool.tile([RD, TQ], BF, tag="rt2")
    nc.vector.scalar_tensor_tensor(t2[:], ps_ap, bias_ap, sin_s[:, sl],
                                   A.add, A.mult)
    # rotate-half of t2 with sign baked in (single-input ops may shift
    # partitions; two-input SBUF ops must share the base partition);
    # bf16 intermediates get the 2x DVE perf mode.
    rot = pool.tile([RD, TQ], BF, tag="rrot")
    nc.vector.tensor_scalar_mul(rot[0:half, :], t2[half:RD, :], -1.0)
    nc.vector.tensor_copy(rot[half:RD, :], t2[0:half, :])
    nc.vector.tensor_add(out_ap, t1[:], rot[:])


# ---------------------------------------------------------------------------
# Host side: shard / preprocess / run / gather
# ---------------------------------------------------------------------------

_cached = {}


def _get_nc(loop=1):
    if loop not in _cached:
        _cached[loop] = build_nc(loop)
    return _cached[loop]


def _pack_dtiles(w, cols):
    """[D, cols] row-major -> [128, ND*cols] with d-tile i at col i*cols."""
    return np.ascontiguousarray(
        w.reshape(ND, 128, cols).transpose(1, 0, 2).reshape(128, ND * cols))


def _prep_inputs(x, Wq, bq, Wkv, bkv, Wkr, bkr, Wku, bku, Wvu, bvu, Wo, bo):
    """Build the 8 per-core input maps."""
    scale = 1.0 / np.sqrt(HD)
    bf = ml_dtypes.bfloat16

    pos = np.arange(T, dtype=np.float64)
    inv_freq = 1.0 / (THETA ** (np.arange(0, RD, 2, dtype=np.float64) / RD))
    ang = pos[:, None] * inv_freq            # (T, 32)
    cosT = np.concatenate([np.cos(ang), np.cos(ang)], -1).T.astype(np.float32)
    sinT = np.concatenate([np.sin(ang), np.sin(ang)], -1).T.astype(np.float32)
    cosT = np.ascontiguousarray(cosT)
    sinT = np.ascontiguousarray(sinT)

    # additive causal mask for the triangular corner of a diagonal tile
    r = np.arange(TKT)[:, None]
    c = np.arange(TKT)[None, :]
    maskneg = np.ascontiguousarray(
        np.where(c >= r, 0.0, NEG).astype(np.float32))

    wkrT = _pack_dtiles(Wkr.T.astype(bf), RD)

    # fold the latent down-projection into the per-head up-projections
    Wkv_k = Wkv[:LAT, :].astype(np.float32)      # (LAT, D)
    Wkv_v = Wkv[LAT:, :].astype(np.float32)
    bkv_k = bkv[:LAT].astype(np.float32)
    bkv_v = bkv[LAT:].astype(np.float32)
    Wk_eff = Wku.astype(np.float32) @ Wkv_k      # (H*CD, D)
    Wv_eff = Wvu.astype(np.float32) @ Wkv_v      # (H*HD, D)
    bk_eff = bku.astype(np.float32) + Wku.astype(np.float32) @ bkv_k

    in_maps = []
    for core in range(NCORES):
        b = core // 4
        hg = core % 4
        he = slice(hg * NH * HD, (hg + 1) * NH * HD)      # 512 q/v dims
        hc = slice(hg * NH * CD, (hg + 1) * NH * CD)      # 256 k-content dims
        biases = np.zeros((128, 21), dtype=np.float32)
        bqh = (bq[he] * scale).reshape(4, 128).T        # [128, head]
        biases[:, 0:4] = bqh
        biases[0:RD, 12] = bkr
        biases[0:RD, 13:17] = bqh[CD:, :]               # rope-row biases
        biases[0:CD, 17:21] = bk_eff[hc].reshape(4, CD).T  # per-head k biases
        xb = np.ascontiguousarray(
            x[b].T.astype(bf).reshape(NXC, 2, 128, T)
            .transpose(0, 2, 1, 3).reshape(NXC, 128, 2 * T))
        in_maps.append({
            "xT": xb,
            "wqT": _pack_dtiles((Wq[he, :] * scale).T.astype(bf), NH * HD),
            "wkT": _pack_dtiles(Wk_eff[hc, :].T.astype(bf), NH * CD),
            "wvT": _pack_dtiles(Wv_eff[he, :].T.astype(bf), NH * HD),
            "wkrT": wkrT,
            "biases": np.ascontiguousarray(biases),
            "woT": np.ascontiguousarray(
                Wo[:, he].T.astype(bf).reshape(NH, 128, D)
                .transpose(1, 0, 2).reshape(128, NH * D)),
            "cosT": cosT,
            "sinT": sinT,
            "maskneg": maskneg,
        })
    return in_maps


def kernel(**inputs):
    inputs = {k: np.asarray(v) for k, v in inputs.items()}
    in_maps = _prep_inputs(**inputs)
    nc = _get_nc(loop=1)
    res = run_bass_kernel_spmd(nc, in_maps, core_ids=list(range(NCORES)))

    Wo, Wvu, Wkv, bkv, bvu, bo = (inputs["Wo"], inputs["Wvu"], inputs["Wkv"],
                                  inputs["bkv"], inputs["bvu"], inputs["bo"])
    bvu_eff = (bvu.astype(np.float64)
               + Wvu.astype(np.float64) @ bkv[LAT:].astype(np.float64))
    const = (Wo.astype(np.float64) @ bvu_eff
             + bo.astype(np.float64)).astype(np.float32)

    out = np.zeros((B, T, D), dtype=np.float32)
    for core in range(NCORES):
        b = core // 4
        # yT packed [NJ, 128, NKT*TQ]: block eo at free offset eo*TQ of
        # column block j  ->  y[eo*128+p, j*TQ+c]
        yT = (res.results[core]["yT"].astype(np.float32)
              .reshape(NJ, 128, NKT, TQ).transpose(2, 1, 0, 3)
              .reshape(D, T))
        out[b] += yT.T
    out += const[None, None, :]
    return out
